# revision 33
# baseline (speedup 1.0000x reference)
"""Contrastive loss (InfoNCE-style) on 8 Trainium2 NeuronCores.

Reference math (B=8192, D=128, temp=0.07):
    sim = (emb @ emb.T) / temp, diag masked to -1e9
    log_probs = log_softmax(sim, axis=1)
    row_mean_i = mean over positives (same label, j != i) of log_probs[i, :]
    loss = -sum(row_mean_i) / count(rows with >=1 positive)

Decomposition used here:
    log_probs[i, j] = sim[i, j] - lse_i,   lse_i = log(sum_{j!=i} exp(sim[i, j]))
    pos_sum_i  = q_i - pc_i * lse_i, where q_i = sum_{j pos} sim[i, j] (exact,
                 computed on host in f64 via class-summed embeddings) and
                 pc_i = (# rows with same label) - 1 (host, exact integer math)
    => the ONLY O(B^2) quantity is esum_i = sum_{j!=i} exp(sim[i, j]).

Perf model (vs the 1.0 s/call baseline): the wall-clock metric is dominated
by the axon tunnel, which has a ~70 ms fixed per-call round-trip cost plus
~16 ms/MB of input transfer; device compute is <1 ms. Three changes attack
exactly that:
  1. Each core receives ONLY its own [128, 1024] embT shard in fp8-e4m3
     (1 MB total, vs 38 MB of per-core rotated f32 copies); the full table
     is rebuilt ON DEVICE with a DRAM AllGather over NeuronLink.
  2. The jitted shard_map executor is built once and cached
     (run_bass_kernel_spmd re-traces + re-jits a fresh closure per call).
  3. The call is dispatched async and the label/einsum host math runs
     during the device round trip.
fp8 numerics: exp-arg jitter ~0.03 -> per-row lse error ~7e-4, final loss
rel err ~1e-4, vs the 2e-2 gate.

Device kernel (per core, SPMD-uniform, no rotation needed):
    - DMA own shard [128, 1024] -> SBUF (lhs source)
    - DRAM bounce + AllGather -> agout [8, 128, 1024]; one multi-dim DMA
      ([c,p,j] -> [p,c,j]) -> embT [128, 8192] in natural global order
    - self-blocks: per row-tile t, matmul lhs_t^T lhs_t -> diag holds raw
      s_ii; affine_select keeps the diagonal (fill -30000), ACT Exp accum
      -> expd[:, t] = exp(s_ii/temp), bit-identical to the diag term inside
      the main sum (same PE/ACT datapath on same operand bits)
    - main: per tile t, 4 quarters x 4 matmuls [128,512] (fp8 -> f32 PSUM),
      ACT Exp(in/temp) with accum_out -> per-quarter row sums
    - output esums [128, 16]: cols 0:8 total exp-sums (incl. self term),
      cols 8:16 exp(diag). Host: esum_excl = total - expd in f64 (exact).

Host: lse = log(esum_excl); row_mean = q/pc - lse (where pc>0); reduce.
"""

import numpy as np

import concourse.bass as bass
import concourse.mybir as mybir
import concourse.tile as tile
from concourse.tile import add_dep_helper
from concourse.bass_utils import run_bass_kernel_spmd

TEMP = 0.07
B = 8192
D = 128
NCORES = 8
RPC = B // NCORES        # 1024 rows per core
NT = RPC // 128          # 8 row-tiles of 128 rows per core
MASK_RAW = -30000.0      # raw-dot space; exp(MASK/temp) == 0.0 in f32

_CACHE = {}

# test.py introspection: last BassKernelResults from run_bass_kernel_spmd.
last_results = None


def _build_bass():
    f32 = mybir.dt.float32
    f8 = mybir.dt.float8e4
    bf16 = mybir.dt.bfloat16
    nc = bass.Bass("TRN2", target_bir_lowering=False, debug=False,
                   num_devices=NCORES)
    eshard = nc.dram_tensor("eshard", [128, RPC], f8, kind="ExternalInput")
    esums = nc.dram_tensor("esums", [128, 2 * NT], f32, kind="ExternalOutput")

    with tile.TileContext(nc) as tc:
        with (
            tc.tile_pool(name="big", bufs=1) as big,
            tc.tile_pool(name="psum", bufs=2, space="PSUM") as psum,
            tc.tile_pool(name="scratch", bufs=32) as scratch,
            tc.tile_pool(name="small", bufs=1) as small,
            tc.tile_pool(name="dram", bufs=1, space="DRAM") as dram,
        ):
            shard_s = big.tile([128, RPC], f8)
            nc.sync.dma_start(out=shard_s[:, :], in_=eshard.ap()[:, :])
            in_dma0 = nc.cur_bb.bb.instructions[-1]
            nc.sync.drain()
            add_dep_helper(nc.cur_bb.bb.instructions[-1], in_dma0, sync=True,
                           reason="observe input DMA queue on SP")

            # AllGather: input bounce (collectives can't touch I/O tensors),
            # gather to a Shared DRAM scratch, then one DMA rebuilds the
            # full [128, 8192] column table in SBUF in natural global order.
            agin = dram.tile([128, RPC], f8)
            agout = dram.tile([NCORES, 128, RPC], f8, addr_space="Shared")
            nc.gpsimd.dma_start(out=agin[:, :], in_=eshard.ap()[:, :])
            agin_dma = nc.cur_bb.bb.instructions[-1]
            nc.gpsimd.collective_compute(
                "AllGather", mybir.AluOpType.bypass,
                replica_groups=[list(range(NCORES))],
                ins=[agin.opt()], outs=[agout.opt()],
            )
            cc_inst = nc.cur_bb.bb.instructions[-1]
            embT = big.tile([128, B], f8)
            # ONE multi-dim DMA for all 8 gathered pieces: walking the DRAM
            # side [c, p, j] -> [p, c, j] lands piece c at SBUF columns
            # [1024c, 1024(c+1)). A single DMA keeps every queue at one
            # entry (walrus allows only one sync wait per DMA entry) and
            # carries the collective wait for the whole gather.
            nc.sync.dma_start(
                out=embT[:, :].rearrange("p (c j) -> p c j", c=NCORES),
                in_=agout[:, :, :].transpose([1, 0, 2]),
            )
            gather_dmas = [nc.cur_bb.bb.instructions[-1]]

            esum_all = small.tile([128, NT * 4], f32)
            esums_s = small.tile([128, 2 * NT], f32)

            # prefetch dummy: a discarded LDWEIGHTS observing the shard DMA,
            # so real matmuls don't carry that queue wait (walrus limit)
            nc.tensor.ldweights(shard_s[:, 0:2].bitcast(bf16))

            # --- self blocks (only need the own shard; overlaps the gather)
            ps_self = psum.tile([128, 2048], f32, tag="ps")
            for t in range(NT):
                lhs = shard_s[:, t * 128:(t + 1) * 128]
                nc.tensor.matmul(ps_self[:, t * 128:(t + 1) * 128], lhs, lhs,
                                 start=True, stop=True)
            # prefetch dummy: a discarded LDWEIGHTS observing the gather DMA
            # on PE, placed after the self matmuls so those still overlap
            # the collective; main matmuls then never carry the gather-queue
            # wait and stay within walrus's one-sync-wait limit
            nc.tensor.ldweights(embT[:, B - 2:B].bitcast(bf16))
            sb_all = small.tile([128, NT * 128], f32)
            nc.scalar.activation(sb_all[:, :], ps_self[:, 0:NT * 128],
                                 mybir.ActivationFunctionType.Copy)
            sbm = small.tile([128, NT * 128], f32)
            nc.gpsimd.affine_select(
                sbm[:, :], sb_all[:, :], pattern=[[0, NT], [-1, 128]],
                compare_op=mybir.AluOpType.is_equal, fill=MASK_RAW,
                base=0, channel_multiplier=1,
            )
            asel_inst = nc.cur_bb.bb.instructions[-1]
            # bf16 like the main-path scratch: the diag's output rounding then
            # matches the main sum's diag term bit-for-bit and cancels exactly
            junkd = small.tile([128, NT * 128], mybir.dt.bfloat16)
            for t in range(NT):
                nc.scalar.activation(
                    junkd[:, t * 128:(t + 1) * 128],
                    sbm[:, t * 128:(t + 1) * 128],
                    mybir.ActivationFunctionType.Exp, scale=1.0 / TEMP,
                    accum_out=esums_s[:, NT + t:NT + t + 1],
                )

            # --- main loop: 8 row-tiles x 4 quarters x 4 matmuls of [128,512]
            for t in range(NT):
                lhs = shard_s[:, t * 128:(t + 1) * 128]
                for q in range(4):
                    qi = t * 4 + q
                    a = qi + 1            # psum alloc index (ps_self was 0)
                    ps = psum.tile([128, 2048], f32, tag="ps")
                    carrier = None
                    if a >= 2:
                        # discarded LDWEIGHTS reading the 2-allocations-ago
                        # ACT result: carries the psum-WAR ACT wait so the
                        # slot-reuse matmul below carries only its own wait
                        obs = (sb_all[:, 0:1] if a == 2
                               else esum_all[:, a - 3:a - 2])
                        nc.tensor.ldweights(obs.bitcast(bf16))
                        carrier = nc.cur_bb.bb.instructions[-1]
                    for k in range(4):
                        n = 4 * q + k
                        nc.tensor.matmul(
                            ps[:, k * 512:(k + 1) * 512],
                            lhs,
                            embT[:, n * 512:(n + 1) * 512],
                            start=True, stop=True,
                        )
                        if carrier is not None:
                            add_dep_helper(nc.cur_bb.bb.instructions[-1],
                                           carrier, sync=False,
                                           reason="wait-carrier order")
                            carrier = None
                        last_mm = nc.cur_bb.bb.instructions[-1]
                    scr = scratch.tile([128, 2048], mybir.dt.bfloat16)
                    nc.scalar.activation(
                        scr[:, :], ps[:, :],
                        mybir.ActivationFunctionType.Exp,
                        scale=1.0 / TEMP,
                        accum_out=esum_all[:, qi:qi + 1],
                    )

            # final [128, 4] -> [128, 1] sums per row-tile on the scalar
            # engine (keeps the vector engine out of the program)
            junk = small.tile([128, 4 * NT], f32)
            for t in range(NT):
                nc.scalar.activation(
                    junk[:, t * 4:(t + 1) * 4],
                    esum_all[:, t * 4:(t + 1) * 4],
                    mybir.ActivationFunctionType.Copy,
                    accum_out=esums_s[:, t:t + 1],
                )
            last_act = nc.cur_bb.bb.instructions[-1]
            # one manual drain per outstanding proc, each carrying a single
            # wait, so the auto-generated kernel-tail drain (which tolerates
            # almost no sync waits) has nothing left to wait for
            nc.sync.drain()
            add_dep_helper(nc.cur_bb.bb.instructions[-1], last_mm, sync=True,
                           reason="observe PE on SP")
            nc.sync.drain()
            add_dep_helper(nc.cur_bb.bb.instructions[-1], last_act, sync=True,
                           reason="observe ACT on SP")
            nc.sync.drain()
            add_dep_helper(nc.cur_bb.bb.instructions[-1], agin_dma, sync=True,
                           reason="observe gpsimd DMA queue on SP")
            nc.sync.drain()
            add_dep_helper(nc.cur_bb.bb.instructions[-1], cc_inst, sync=True,
                           reason="observe collective on SP")
            nc.sync.drain()
            add_dep_helper(nc.cur_bb.bb.instructions[-1], asel_inst, sync=True,
                           reason="observe gpsimd engine on SP")
            for g in gather_dmas:
                nc.sync.drain()
                add_dep_helper(nc.cur_bb.bb.instructions[-1], g, sync=True,
                               reason="observe gather DMA queue on SP")
            nc.sync.dma_start(out=esums.ap()[:, :], in_=esums_s[:, :])
            out_dma = nc.cur_bb.bb.instructions[-1]
            nc.sync.drain()
            add_dep_helper(nc.cur_bb.bb.instructions[-1], out_dma, sync=True,
                           reason="observe out DMA queue on SP")
    return nc


def _get_nc():
    if "nc" not in _CACHE:
        _CACHE["nc"] = _build_bass()
    return _CACHE["nc"]


def _host_inputs(emb):
    """Per-core in_maps: just the core's own [128, 1024] embT shard (fp8)."""
    import ml_dtypes
    return [{"eshard": emb[RPC * c:RPC * (c + 1)].T.astype(ml_dtypes.float8_e4m3)}
            for c in range(NCORES)]


def _get_exec():
    """Build ONCE a jitted shard_map executor for the bass program.

    run_bass_kernel_spmd re-creates and re-jits its closure on every call
    (fresh trace + lower each time); since the wall-clock metric is
    dominated by per-call dispatch, cache the jitted callable and reuse it.
    Mirrors bass2jax.run_bass_via_pjrt's multi-core path exactly.
    """
    if "exec" in _CACHE:
        return _CACHE["exec"]
    import jax
    from jax.sharding import Mesh, PartitionSpec
    import warnings
    with warnings.catch_warnings():
        warnings.simplefilter("ignore")
        from jax.experimental.shard_map import shard_map
    from concourse.bass2jax import (
        install_neuronx_cc_hook, _bass_exec_p, partition_id_tensor,
    )

    nc = _get_nc()
    install_neuronx_cc_hook()
    assert nc.dbg_addr is None
    partition_name = (nc.partition_id_tensor.name
                      if nc.partition_id_tensor else None)
    in_names, out_names, out_avals = [], [], []
    for alloc in nc.m.functions[0].allocations:
        if not isinstance(alloc, mybir.MemoryLocationSet):
            continue
        name = alloc.memorylocations[0].name
        if alloc.kind == "ExternalInput":
            if name != partition_name:
                in_names.append(name)
        elif alloc.kind == "ExternalOutput":
            out_names.append(name)
            out_avals.append(jax.core.ShapedArray(
                tuple(alloc.tensor_shape), mybir.dt.np(alloc.dtype)))
    assert in_names == ["eshard"] and out_names == ["esums"]
    n_params = len(in_names)
    in_names_full = in_names + out_names
    if partition_name is not None:
        in_names_full.append(partition_name)
    donate = tuple(range(n_params, n_params + len(out_names)))

    def _body(*args):
        operands = list(args)
        if partition_name is not None:
            operands.append(partition_id_tensor())
        return tuple(_bass_exec_p.bind(
            *operands,
            out_avals=tuple(out_avals),
            in_names=tuple(in_names_full),
            out_names=tuple(out_names),
            lowering_input_output_aliases=(),
            sim_require_finite=True,
            sim_require_nnan=True,
            nc=nc,
        ))

    devices = jax.devices()[:NCORES]
    assert len(devices) == NCORES
    mesh = Mesh(np.asarray(devices), ("core",))
    nio = n_params + len(out_names)
    sharded = jax.jit(
        shard_map(_body, mesh=mesh,
                  in_specs=(PartitionSpec("core"),) * nio,
                  out_specs=(PartitionSpec("core"),) * len(out_names),
                  check_rep=False),
        donate_argnums=donate, keep_unused=True,
    )
    try:
        import ml_dtypes
        sharded = sharded.lower(
            jax.ShapeDtypeStruct((NCORES * 128, RPC), ml_dtypes.float8_e4m3),
            jax.ShapeDtypeStruct((NCORES * 128, 2 * NT), np.float32),
        ).compile()
    except Exception:
        pass  # plain jit callable works too, just ~1 ms more dispatch
    _CACHE["exec"] = sharded
    return sharded


def _run_spmd_fallback(emb):
    """Reference path through run_bass_kernel_spmd (per-call re-jit)."""
    global last_results
    res = run_bass_kernel_spmd(_get_nc(), _host_inputs(emb),
                               core_ids=list(range(NCORES)))
    last_results = res
    return np.stack([np.asarray(res.results[c]["esums"])
                     for c in range(NCORES)])


def kernel(embeddings, labels):
    emb = np.ascontiguousarray(np.asarray(embeddings, dtype=np.float32))
    labels = np.asarray(labels).astype(np.int64)
    assert emb.shape == (B, D) and labels.shape == (B,)

    # stacked shards [8*128, 1024] fp8: rows [128c:128(c+1)] = shard c, T'd.
    # f32 -> bf16 bits via integer round-to-nearest-even (3 vector ops),
    # then a 64K LUT bf16-pattern -> e4m3 byte: ~6 ms vs ~9 ms for
    # ml_dtypes' direct f32->e4m3 cast. Must ROUND to bf16, not truncate:
    # truncation's round-toward-zero bias costs 10x final accuracy.
    import ml_dtypes
    if "f8lut" not in _CACHE:
        with np.errstate(invalid="ignore", over="ignore"):
            _CACHE["f8lut"] = ((np.arange(65536, dtype=np.uint32) << 16)
                               .view(np.float32)
                               .astype(ml_dtypes.float8_e4m3)
                               .view(np.uint8))
    bits = emb.view(np.uint32)
    idx = ((bits + np.uint32(0x7FFF) + ((bits >> np.uint32(16)) & np.uint32(1)))
           >> np.uint32(16)).astype(np.uint16)
    stacked = (_CACHE["f8lut"][idx.reshape(NCORES, RPC, D).transpose(0, 2, 1)]
               .view(ml_dtypes.float8_e4m3).reshape(NCORES * 128, RPC))
    out_fut = None
    try:
        sharded = _get_exec()
        out_fut = sharded(stacked, np.zeros((NCORES * 128, 2 * NT), np.float32))
    except Exception:
        outs3 = _run_spmd_fallback(emb)

    # overlap the label/einsum host math with the device round trip
    emb64 = emb.astype(np.float64)
    nclass = int(labels.max()) + 1
    cnt = np.bincount(labels, minlength=nclass)
    pc = cnt[labels] - 1                      # positives per row (excl. self)
    G = np.zeros((nclass, D), dtype=np.float64)
    np.add.at(G, labels, emb64)
    # q_i = sum over positives j (same label, j != i) of sim[i, j]
    q = (np.einsum("ij,ij->i", emb64, G[labels])
         - np.einsum("ij,ij->i", emb64, emb64)) / TEMP

    if out_fut is not None:
        outs3 = np.asarray(out_fut[0]).reshape(NCORES, 128, 2 * NT)

    # esums[p, 0:8] full exp-sum, [p, 8:16] exp(diag); local row = 128*t + p
    tot = np.ascontiguousarray(
        outs3[:, :, :NT].transpose(0, 2, 1)).reshape(-1).astype(np.float64)
    expd = np.ascontiguousarray(
        outs3[:, :, NT:].transpose(0, 2, 1)).reshape(-1).astype(np.float64)
    esum = tot - expd

    lse = np.log(esum)
    has = pc > 0
    row_mean = np.where(has, q / np.maximum(pc, 1) - lse, 0.0)
    loss = -row_mean.sum() / max(int(has.sum()), 1)
    return np.float32(loss)


# revision 34
# speedup vs baseline: 1.0379x; 1.0379x over previous
"""Contrastive loss (InfoNCE-style) on 8 Trainium2 NeuronCores.

Reference math (B=8192, D=128, temp=0.07):
    sim = (emb @ emb.T) / temp, diag masked to -1e9
    log_probs = log_softmax(sim, axis=1)
    row_mean_i = mean over positives (same label, j != i) of log_probs[i, :]
    loss = -sum(row_mean_i) / count(rows with >=1 positive)

Decomposition used here:
    log_probs[i, j] = sim[i, j] - lse_i,   lse_i = log(sum_{j!=i} exp(sim[i, j]))
    pos_sum_i  = q_i - pc_i * lse_i, where q_i = sum_{j pos} sim[i, j] (exact,
                 computed on host in f64 via class-summed embeddings) and
                 pc_i = (# rows with same label) - 1 (host, exact integer math)
    => the ONLY O(B^2) quantity is esum_i = sum_{j!=i} exp(sim[i, j]).

Perf model (vs the 1.0 s/call baseline): the wall-clock metric is dominated
by the axon tunnel, which has a ~70 ms fixed per-call round-trip cost plus
~16 ms/MB of input transfer; device compute is <1 ms. Three changes attack
exactly that:
  1. Each core receives ONLY its own [128, 1024] embT shard in fp8-e4m3
     (1 MB total, vs 38 MB of per-core rotated f32 copies); the full table
     is rebuilt ON DEVICE with a DRAM AllGather over NeuronLink.
  2. The jitted shard_map executor is built once and cached
     (run_bass_kernel_spmd re-traces + re-jits a fresh closure per call).
  3. The call is dispatched async and the label/einsum host math runs
     during the device round trip.
fp8 numerics: exp-arg jitter ~0.03 -> per-row lse error ~7e-4, final loss
rel err ~1e-4, vs the 2e-2 gate.

Device kernel (per core, SPMD-uniform, no rotation needed):
    - DMA own shard [128, 1024] -> SBUF (lhs source)
    - DRAM bounce + AllGather -> agout [8, 128, 1024]; one multi-dim DMA
      ([c,p,j] -> [p,c,j]) -> embT [128, 8192] in natural global order
    - self-blocks: per row-tile t, matmul lhs_t^T lhs_t -> diag holds raw
      s_ii; affine_select keeps the diagonal (fill -30000), ACT Exp accum
      -> expd[:, t] = exp(s_ii/temp), bit-identical to the diag term inside
      the main sum (same PE/ACT datapath on same operand bits)
    - main: per tile t, 4 quarters x 4 matmuls [128,512] (fp8 -> f32 PSUM),
      ACT Exp(in/temp) with accum_out -> per-quarter row sums
    - output esums [128, 16]: cols 0:8 total exp-sums (incl. self term),
      cols 8:16 exp(diag). Host: esum_excl = total - expd in f64 (exact).

Host: lse = log(esum_excl); row_mean = q/pc - lse (where pc>0); reduce.
"""

import numpy as np

import concourse.bass as bass
import concourse.mybir as mybir
import concourse.tile as tile
from concourse.tile import add_dep_helper
from concourse.bass_utils import run_bass_kernel_spmd

TEMP = 0.07
B = 8192
D = 128
NCORES = 8
RPC = B // NCORES        # 1024 rows per core
NT = RPC // 128          # 8 row-tiles of 128 rows per core
MASK_RAW = -30000.0      # raw-dot space; exp(MASK/temp) == 0.0 in f32

_CACHE = {}

# test.py introspection: last BassKernelResults from run_bass_kernel_spmd.
last_results = None


def _build_bass():
    f32 = mybir.dt.float32
    f8 = mybir.dt.float8e4
    bf16 = mybir.dt.bfloat16
    nc = bass.Bass("TRN2", target_bir_lowering=False, debug=False,
                   num_devices=NCORES)
    eshard = nc.dram_tensor("eshard", [128, RPC], f8, kind="ExternalInput")
    esums = nc.dram_tensor("esums", [128, 2 * NT], f32, kind="ExternalOutput")

    with tile.TileContext(nc) as tc:
        with (
            tc.tile_pool(name="big", bufs=1) as big,
            tc.tile_pool(name="psum", bufs=2, space="PSUM") as psum,
            tc.tile_pool(name="scratch", bufs=32) as scratch,
            tc.tile_pool(name="small", bufs=1) as small,
            tc.tile_pool(name="dram", bufs=1, space="DRAM") as dram,
        ):
            shard_s = big.tile([128, RPC], f8)
            nc.sync.dma_start(out=shard_s[:, :], in_=eshard.ap()[:, :])
            in_dma0 = nc.cur_bb.bb.instructions[-1]
            nc.sync.drain()
            add_dep_helper(nc.cur_bb.bb.instructions[-1], in_dma0, sync=True,
                           reason="observe input DMA queue on SP")

            # AllGather: input bounce (collectives can't touch I/O tensors),
            # gather to a Shared DRAM scratch, then one DMA rebuilds the
            # full [128, 8192] column table in SBUF in natural global order.
            agin = dram.tile([128, RPC], f8)
            agout = dram.tile([NCORES, 128, RPC], f8, addr_space="Shared")
            nc.gpsimd.dma_start(out=agin[:, :], in_=eshard.ap()[:, :])
            agin_dma = nc.cur_bb.bb.instructions[-1]
            nc.gpsimd.collective_compute(
                "AllGather", mybir.AluOpType.bypass,
                replica_groups=[list(range(NCORES))],
                ins=[agin.opt()], outs=[agout.opt()],
            )
            cc_inst = nc.cur_bb.bb.instructions[-1]
            embT = big.tile([128, B], f8)
            # ONE multi-dim DMA for all 8 gathered pieces: walking the DRAM
            # side [c, p, j] -> [p, c, j] lands piece c at SBUF columns
            # [1024c, 1024(c+1)). A single DMA keeps every queue at one
            # entry (walrus allows only one sync wait per DMA entry) and
            # carries the collective wait for the whole gather.
            nc.sync.dma_start(
                out=embT[:, :].rearrange("p (c j) -> p c j", c=NCORES),
                in_=agout[:, :, :].transpose([1, 0, 2]),
            )
            gather_dmas = [nc.cur_bb.bb.instructions[-1]]

            esum_all = small.tile([128, NT * 4], f32)
            esums_s = small.tile([128, 2 * NT], f32)

            # prefetch dummy: a discarded LDWEIGHTS observing the shard DMA,
            # so real matmuls don't carry that queue wait (walrus limit)
            nc.tensor.ldweights(shard_s[:, 0:2].bitcast(bf16))

            # --- self blocks (only need the own shard; overlaps the gather)
            ps_self = psum.tile([128, 2048], f32, tag="ps")
            for t in range(NT):
                lhs = shard_s[:, t * 128:(t + 1) * 128]
                nc.tensor.matmul(ps_self[:, t * 128:(t + 1) * 128], lhs, lhs,
                                 start=True, stop=True)
            # prefetch dummy: a discarded LDWEIGHTS observing the gather DMA
            # on PE, placed after the self matmuls so those still overlap
            # the collective; main matmuls then never carry the gather-queue
            # wait and stay within walrus's one-sync-wait limit
            nc.tensor.ldweights(embT[:, B - 2:B].bitcast(bf16))
            sb_all = small.tile([128, NT * 128], f32)
            nc.scalar.activation(sb_all[:, :], ps_self[:, 0:NT * 128],
                                 mybir.ActivationFunctionType.Copy)
            sbm = small.tile([128, NT * 128], f32)
            nc.gpsimd.affine_select(
                sbm[:, :], sb_all[:, :], pattern=[[0, NT], [-1, 128]],
                compare_op=mybir.AluOpType.is_equal, fill=MASK_RAW,
                base=0, channel_multiplier=1,
            )
            asel_inst = nc.cur_bb.bb.instructions[-1]
            # bf16 like the main-path scratch: the diag's output rounding then
            # matches the main sum's diag term bit-for-bit and cancels exactly
            junkd = small.tile([128, NT * 128], mybir.dt.bfloat16)
            for t in range(NT):
                nc.scalar.activation(
                    junkd[:, t * 128:(t + 1) * 128],
                    sbm[:, t * 128:(t + 1) * 128],
                    mybir.ActivationFunctionType.Exp, scale=1.0 / TEMP,
                    accum_out=esums_s[:, NT + t:NT + t + 1],
                )

            # --- main loop: 8 row-tiles x 4 quarters x 4 matmuls of [128,512]
            for t in range(NT):
                lhs = shard_s[:, t * 128:(t + 1) * 128]
                for q in range(4):
                    qi = t * 4 + q
                    a = qi + 1            # psum alloc index (ps_self was 0)
                    ps = psum.tile([128, 2048], f32, tag="ps")
                    carrier = None
                    if a >= 2:
                        # discarded LDWEIGHTS reading the 2-allocations-ago
                        # ACT result: carries the psum-WAR ACT wait so the
                        # slot-reuse matmul below carries only its own wait
                        obs = (sb_all[:, 0:1] if a == 2
                               else esum_all[:, a - 3:a - 2])
                        nc.tensor.ldweights(obs.bitcast(bf16))
                        carrier = nc.cur_bb.bb.instructions[-1]
                    for k in range(4):
                        n = 4 * q + k
                        nc.tensor.matmul(
                            ps[:, k * 512:(k + 1) * 512],
                            lhs,
                            embT[:, n * 512:(n + 1) * 512],
                            start=True, stop=True,
                        )
                        if carrier is not None:
                            add_dep_helper(nc.cur_bb.bb.instructions[-1],
                                           carrier, sync=False,
                                           reason="wait-carrier order")
                            carrier = None
                        last_mm = nc.cur_bb.bb.instructions[-1]
                    scr = scratch.tile([128, 2048], mybir.dt.bfloat16)
                    nc.scalar.activation(
                        scr[:, :], ps[:, :],
                        mybir.ActivationFunctionType.Exp,
                        scale=1.0 / TEMP,
                        accum_out=esum_all[:, qi:qi + 1],
                    )

            # final [128, 4] -> [128, 1] sums per row-tile on the scalar
            # engine (keeps the vector engine out of the program)
            junk = small.tile([128, 4 * NT], f32)
            for t in range(NT):
                nc.scalar.activation(
                    junk[:, t * 4:(t + 1) * 4],
                    esum_all[:, t * 4:(t + 1) * 4],
                    mybir.ActivationFunctionType.Copy,
                    accum_out=esums_s[:, t:t + 1],
                )
            last_act = nc.cur_bb.bb.instructions[-1]
            # one manual drain per outstanding proc, each carrying a single
            # wait, so the auto-generated kernel-tail drain (which tolerates
            # almost no sync waits) has nothing left to wait for
            nc.sync.drain()
            add_dep_helper(nc.cur_bb.bb.instructions[-1], last_mm, sync=True,
                           reason="observe PE on SP")
            nc.sync.drain()
            add_dep_helper(nc.cur_bb.bb.instructions[-1], last_act, sync=True,
                           reason="observe ACT on SP")
            nc.sync.drain()
            add_dep_helper(nc.cur_bb.bb.instructions[-1], agin_dma, sync=True,
                           reason="observe gpsimd DMA queue on SP")
            nc.sync.drain()
            add_dep_helper(nc.cur_bb.bb.instructions[-1], cc_inst, sync=True,
                           reason="observe collective on SP")
            nc.sync.drain()
            add_dep_helper(nc.cur_bb.bb.instructions[-1], asel_inst, sync=True,
                           reason="observe gpsimd engine on SP")
            for g in gather_dmas:
                nc.sync.drain()
                add_dep_helper(nc.cur_bb.bb.instructions[-1], g, sync=True,
                               reason="observe gather DMA queue on SP")
            nc.sync.dma_start(out=esums.ap()[:, :], in_=esums_s[:, :])
            out_dma = nc.cur_bb.bb.instructions[-1]
            nc.sync.drain()
            add_dep_helper(nc.cur_bb.bb.instructions[-1], out_dma, sync=True,
                           reason="observe out DMA queue on SP")
    return nc


def _get_nc():
    if "nc" not in _CACHE:
        _CACHE["nc"] = _build_bass()
    return _CACHE["nc"]


def _host_inputs(emb):
    """Per-core in_maps: just the core's own [128, 1024] embT shard (fp8)."""
    import ml_dtypes
    return [{"eshard": emb[RPC * c:RPC * (c + 1)].T.astype(ml_dtypes.float8_e4m3)}
            for c in range(NCORES)]


def _get_exec():
    """Build ONCE a jitted shard_map executor for the bass program.

    run_bass_kernel_spmd re-creates and re-jits its closure on every call
    (fresh trace + lower each time); since the wall-clock metric is
    dominated by per-call dispatch, cache the jitted callable and reuse it.
    Mirrors bass2jax.run_bass_via_pjrt's multi-core path exactly.
    """
    if "exec" in _CACHE:
        return _CACHE["exec"]
    import jax
    from jax.sharding import Mesh, PartitionSpec
    import warnings
    with warnings.catch_warnings():
        warnings.simplefilter("ignore")
        from jax.experimental.shard_map import shard_map
    from concourse.bass2jax import (
        install_neuronx_cc_hook, _bass_exec_p, partition_id_tensor,
    )

    nc = _get_nc()
    install_neuronx_cc_hook()
    assert nc.dbg_addr is None
    partition_name = (nc.partition_id_tensor.name
                      if nc.partition_id_tensor else None)
    in_names, out_names, out_avals = [], [], []
    for alloc in nc.m.functions[0].allocations:
        if not isinstance(alloc, mybir.MemoryLocationSet):
            continue
        name = alloc.memorylocations[0].name
        if alloc.kind == "ExternalInput":
            if name != partition_name:
                in_names.append(name)
        elif alloc.kind == "ExternalOutput":
            out_names.append(name)
            out_avals.append(jax.core.ShapedArray(
                tuple(alloc.tensor_shape), mybir.dt.np(alloc.dtype)))
    assert in_names == ["eshard"] and out_names == ["esums"]
    n_params = len(in_names)
    in_names_full = in_names + out_names
    if partition_name is not None:
        in_names_full.append(partition_name)
    donate = tuple(range(n_params, n_params + len(out_names)))

    def _body(*args):
        operands = list(args)
        if partition_name is not None:
            operands.append(partition_id_tensor())
        return tuple(_bass_exec_p.bind(
            *operands,
            out_avals=tuple(out_avals),
            in_names=tuple(in_names_full),
            out_names=tuple(out_names),
            lowering_input_output_aliases=(),
            sim_require_finite=True,
            sim_require_nnan=True,
            nc=nc,
        ))

    devices = jax.devices()[:NCORES]
    assert len(devices) == NCORES
    mesh = Mesh(np.asarray(devices), ("core",))
    nio = n_params + len(out_names)
    sharded = jax.jit(
        shard_map(_body, mesh=mesh,
                  in_specs=(PartitionSpec("core"),) * nio,
                  out_specs=(PartitionSpec("core"),) * len(out_names),
                  check_rep=False),
        donate_argnums=donate, keep_unused=True,
    )
    try:
        import ml_dtypes
        sharded = sharded.lower(
            jax.ShapeDtypeStruct((NCORES * 128, RPC), ml_dtypes.float8_e4m3),
            jax.ShapeDtypeStruct((NCORES * 128, 2 * NT), np.float32),
        ).compile()
    except Exception:
        pass  # plain jit callable works too, just ~1 ms more dispatch
    _CACHE["exec"] = sharded
    return sharded


def _run_spmd_fallback(emb):
    """Reference path through run_bass_kernel_spmd (per-call re-jit)."""
    global last_results
    res = run_bass_kernel_spmd(_get_nc(), _host_inputs(emb),
                               core_ids=list(range(NCORES)))
    last_results = res
    return np.stack([np.asarray(res.results[c]["esums"])
                     for c in range(NCORES)])


def kernel(embeddings, labels):
    emb = np.ascontiguousarray(np.asarray(embeddings, dtype=np.float32))
    labels = np.asarray(labels).astype(np.int64)
    assert emb.shape == (B, D) and labels.shape == (B,)

    # stacked shards [8*128, 1024] fp8: rows [128c:128(c+1)] = shard c, T'd.
    # f32 -> f16 (round-to-nearest numpy cast) -> e4m3 via a 64K LUT on
    # the f16 bits: ~6 ms vs ~9 ms for ml_dtypes' direct f32->e4m3 cast.
    # (A bf16-indexed LUT is ~1 ms cheaper but the coarser intermediate
    # costs 1.6x accuracy; f32-top-16 truncation costs 10x. Neither is
    # worth it against run-to-run tunnel variance.)
    import ml_dtypes
    if "f8lut" not in _CACHE:
        with np.errstate(invalid="ignore", over="ignore"):
            _CACHE["f8lut"] = (np.arange(65536, dtype=np.uint16)
                               .view(np.float16)
                               .astype(ml_dtypes.float8_e4m3)
                               .view(np.uint8))
    s16 = emb.reshape(NCORES, RPC, D).transpose(0, 2, 1).astype(np.float16)
    stacked = (_CACHE["f8lut"][s16.view(np.uint16)]
               .view(ml_dtypes.float8_e4m3).reshape(NCORES * 128, RPC))
    out_fut = None
    try:
        sharded = _get_exec()
        out_fut = sharded(stacked, np.zeros((NCORES * 128, 2 * NT), np.float32))
    except Exception:
        outs3 = _run_spmd_fallback(emb)

    # overlap the label/einsum host math with the device round trip
    emb64 = emb.astype(np.float64)
    nclass = int(labels.max()) + 1
    cnt = np.bincount(labels, minlength=nclass)
    pc = cnt[labels] - 1                      # positives per row (excl. self)
    G = np.zeros((nclass, D), dtype=np.float64)
    np.add.at(G, labels, emb64)
    # q_i = sum over positives j (same label, j != i) of sim[i, j]
    q = (np.einsum("ij,ij->i", emb64, G[labels])
         - np.einsum("ij,ij->i", emb64, emb64)) / TEMP

    if out_fut is not None:
        outs3 = np.asarray(out_fut[0]).reshape(NCORES, 128, 2 * NT)

    # esums[p, 0:8] full exp-sum, [p, 8:16] exp(diag); local row = 128*t + p
    tot = np.ascontiguousarray(
        outs3[:, :, :NT].transpose(0, 2, 1)).reshape(-1).astype(np.float64)
    expd = np.ascontiguousarray(
        outs3[:, :, NT:].transpose(0, 2, 1)).reshape(-1).astype(np.float64)
    esum = tot - expd

    lse = np.log(esum)
    has = pc > 0
    row_mean = np.where(has, q / np.maximum(pc, 1) - lse, 0.0)
    loss = -row_mean.sum() / max(int(has.sum()), 1)
    return np.float32(loss)


# revision 36
# speedup vs baseline: 1.0688x; 1.0298x over previous
"""Contrastive loss (InfoNCE-style) on 8 Trainium2 NeuronCores.

Reference math (B=8192, D=128, temp=0.07):
    sim = (emb @ emb.T) / temp, diag masked to -1e9
    log_probs = log_softmax(sim, axis=1)
    row_mean_i = mean over positives (same label, j != i) of log_probs[i, :]
    loss = -sum(row_mean_i) / count(rows with >=1 positive)

Decomposition used here:
    log_probs[i, j] = sim[i, j] - lse_i,   lse_i = log(sum_{j!=i} exp(sim[i, j]))
    pos_sum_i  = q_i - pc_i * lse_i, where q_i = sum_{j pos} sim[i, j] (exact,
                 computed on host in f64 via class-summed embeddings) and
                 pc_i = (# rows with same label) - 1 (host, exact integer math)
    => the ONLY O(B^2) quantity is esum_i = sum_{j!=i} exp(sim[i, j]).

Perf model (vs the 1.0 s/call baseline): the wall-clock metric is dominated
by the axon tunnel, which has a ~70 ms fixed per-call round-trip cost plus
~16 ms/MB of input transfer; device compute is <1 ms. Three changes attack
exactly that:
  1. Each core receives ONLY its own [128, 1024] embT shard in fp8-e4m3
     (1 MB total, vs 38 MB of per-core rotated f32 copies); the full table
     is rebuilt ON DEVICE with a DRAM AllGather over NeuronLink.
  2. The jitted shard_map executor is built once and cached
     (run_bass_kernel_spmd re-traces + re-jits a fresh closure per call).
  3. The call is dispatched async and the label/einsum host math runs
     during the device round trip.
fp8 numerics: exp-arg jitter ~0.03 -> per-row lse error ~7e-4, final loss
rel err ~1e-4, vs the 2e-2 gate.

Device kernel (per core, SPMD-uniform, no rotation needed):
    - DMA own shard [128, 1024] -> SBUF (lhs source)
    - DRAM bounce + AllGather -> agout [8, 128, 1024]; one multi-dim DMA
      ([c,p,j] -> [p,c,j]) -> embT [128, 8192] in natural global order
    - self-blocks: per row-tile t, matmul lhs_t^T lhs_t -> diag holds raw
      s_ii; affine_select keeps the diagonal (fill -30000), ACT Exp accum
      -> expd[:, t] = exp(s_ii/temp), bit-identical to the diag term inside
      the main sum (same PE/ACT datapath on same operand bits)
    - main: per tile t, 4 quarters x 4 matmuls [128,512] (fp8 -> f32 PSUM),
      ACT Exp(in/temp) with accum_out -> per-quarter row sums
    - output esums [128, 16]: cols 0:8 total exp-sums (incl. self term),
      cols 8:16 exp(diag). Host: esum_excl = total - expd in f64 (exact).

Host: lse = log(esum_excl); row_mean = q/pc - lse (where pc>0); reduce.
"""

import numpy as np

import concourse.bass as bass
import concourse.mybir as mybir
import concourse.tile as tile
from concourse.tile import add_dep_helper
from concourse.bass_utils import run_bass_kernel_spmd

TEMP = 0.07
B = 8192
D = 128
NCORES = 8
RPC = B // NCORES        # 1024 rows per core
NT = RPC // 128          # 8 row-tiles of 128 rows per core
MASK_RAW = -30000.0      # raw-dot space; exp(MASK/temp) == 0.0 in f32

_CACHE = {}

# test.py introspection: last BassKernelResults from run_bass_kernel_spmd.
last_results = None


def _build_bass():
    f32 = mybir.dt.float32
    f8 = mybir.dt.float8e4
    bf16 = mybir.dt.bfloat16
    nc = bass.Bass("TRN2", target_bir_lowering=False, debug=False,
                   num_devices=NCORES)
    eshard = nc.dram_tensor("eshard", [128, RPC], f8, kind="ExternalInput")
    esums = nc.dram_tensor("esums", [128, 2 * NT], f32, kind="ExternalOutput")

    with tile.TileContext(nc) as tc:
        with (
            tc.tile_pool(name="big", bufs=1) as big,
            tc.tile_pool(name="psum", bufs=2, space="PSUM") as psum,
            tc.tile_pool(name="scratch", bufs=32) as scratch,
            tc.tile_pool(name="small", bufs=1) as small,
            tc.tile_pool(name="dram", bufs=1, space="DRAM") as dram,
        ):
            shard_s = big.tile([128, RPC], f8)
            nc.sync.dma_start(out=shard_s[:, :], in_=eshard.ap()[:, :])
            in_dma0 = nc.cur_bb.bb.instructions[-1]
            nc.sync.drain()
            add_dep_helper(nc.cur_bb.bb.instructions[-1], in_dma0, sync=True,
                           reason="observe input DMA queue on SP")

            # AllGather: input bounce (collectives can't touch I/O tensors),
            # gather to a Shared DRAM scratch, then one DMA rebuilds the
            # full [128, 8192] column table in SBUF in natural global order.
            agin = dram.tile([128, RPC], f8)
            agout = dram.tile([NCORES, 128, RPC], f8, addr_space="Shared")
            nc.gpsimd.dma_start(out=agin[:, :], in_=eshard.ap()[:, :])
            agin_dma = nc.cur_bb.bb.instructions[-1]
            nc.gpsimd.collective_compute(
                "AllGather", mybir.AluOpType.bypass,
                replica_groups=[list(range(NCORES))],
                ins=[agin.opt()], outs=[agout.opt()],
            )
            cc_inst = nc.cur_bb.bb.instructions[-1]
            embT = big.tile([128, B], f8)
            # ONE multi-dim DMA for all 8 gathered pieces: walking the DRAM
            # side [c, p, j] -> [p, c, j] lands piece c at SBUF columns
            # [1024c, 1024(c+1)). A single DMA keeps every queue at one
            # entry (walrus allows only one sync wait per DMA entry) and
            # carries the collective wait for the whole gather.
            nc.sync.dma_start(
                out=embT[:, :].rearrange("p (c j) -> p c j", c=NCORES),
                in_=agout[:, :, :].transpose([1, 0, 2]),
            )
            gather_dmas = [nc.cur_bb.bb.instructions[-1]]

            esum_all = small.tile([128, NT * 4], f32)
            esums_s = small.tile([128, 2 * NT], f32)

            # prefetch dummy: a discarded LDWEIGHTS observing the shard DMA,
            # so real matmuls don't carry that queue wait (walrus limit)
            nc.tensor.ldweights(shard_s[:, 0:2].bitcast(bf16))

            # --- self blocks (only need the own shard; overlaps the gather)
            ps_self = psum.tile([128, 2048], f32, tag="ps")
            for t in range(NT):
                lhs = shard_s[:, t * 128:(t + 1) * 128]
                nc.tensor.matmul(ps_self[:, t * 128:(t + 1) * 128], lhs, lhs,
                                 start=True, stop=True)
            # prefetch dummy: a discarded LDWEIGHTS observing the gather DMA
            # on PE, placed after the self matmuls so those still overlap
            # the collective; main matmuls then never carry the gather-queue
            # wait and stay within walrus's one-sync-wait limit
            nc.tensor.ldweights(embT[:, B - 2:B].bitcast(bf16))
            sb_all = small.tile([128, NT * 128], f32)
            nc.scalar.activation(sb_all[:, :], ps_self[:, 0:NT * 128],
                                 mybir.ActivationFunctionType.Copy)
            sbm = small.tile([128, NT * 128], f32)
            nc.gpsimd.affine_select(
                sbm[:, :], sb_all[:, :], pattern=[[0, NT], [-1, 128]],
                compare_op=mybir.AluOpType.is_equal, fill=MASK_RAW,
                base=0, channel_multiplier=1,
            )
            asel_inst = nc.cur_bb.bb.instructions[-1]
            # bf16 like the main-path scratch: the diag's output rounding then
            # matches the main sum's diag term bit-for-bit and cancels exactly
            junkd = small.tile([128, NT * 128], mybir.dt.bfloat16)
            for t in range(NT):
                nc.scalar.activation(
                    junkd[:, t * 128:(t + 1) * 128],
                    sbm[:, t * 128:(t + 1) * 128],
                    mybir.ActivationFunctionType.Exp, scale=1.0 / TEMP,
                    accum_out=esums_s[:, NT + t:NT + t + 1],
                )

            # --- main loop: 8 row-tiles x 4 quarters x 4 matmuls of [128,512]
            for t in range(NT):
                lhs = shard_s[:, t * 128:(t + 1) * 128]
                for q in range(4):
                    qi = t * 4 + q
                    a = qi + 1            # psum alloc index (ps_self was 0)
                    ps = psum.tile([128, 2048], f32, tag="ps")
                    carrier = None
                    if a >= 2:
                        # discarded LDWEIGHTS reading the 2-allocations-ago
                        # ACT result: carries the psum-WAR ACT wait so the
                        # slot-reuse matmul below carries only its own wait
                        obs = (sb_all[:, 0:1] if a == 2
                               else esum_all[:, a - 3:a - 2])
                        nc.tensor.ldweights(obs.bitcast(bf16))
                        carrier = nc.cur_bb.bb.instructions[-1]
                    for k in range(4):
                        n = 4 * q + k
                        nc.tensor.matmul(
                            ps[:, k * 512:(k + 1) * 512],
                            lhs,
                            embT[:, n * 512:(n + 1) * 512],
                            start=True, stop=True,
                        )
                        if carrier is not None:
                            add_dep_helper(nc.cur_bb.bb.instructions[-1],
                                           carrier, sync=False,
                                           reason="wait-carrier order")
                            carrier = None
                        last_mm = nc.cur_bb.bb.instructions[-1]
                    scr = scratch.tile([128, 2048], mybir.dt.bfloat16)
                    nc.scalar.activation(
                        scr[:, :], ps[:, :],
                        mybir.ActivationFunctionType.Exp,
                        scale=1.0 / TEMP,
                        accum_out=esum_all[:, qi:qi + 1],
                    )

            # final [128, 4] -> [128, 1] sums per row-tile on the scalar
            # engine (keeps the vector engine out of the program)
            junk = small.tile([128, 4 * NT], f32)
            for t in range(NT):
                nc.scalar.activation(
                    junk[:, t * 4:(t + 1) * 4],
                    esum_all[:, t * 4:(t + 1) * 4],
                    mybir.ActivationFunctionType.Copy,
                    accum_out=esums_s[:, t:t + 1],
                )
            last_act = nc.cur_bb.bb.instructions[-1]
            # one manual drain per outstanding proc, each carrying a single
            # wait, so the auto-generated kernel-tail drain (which tolerates
            # almost no sync waits) has nothing left to wait for
            nc.sync.drain()
            add_dep_helper(nc.cur_bb.bb.instructions[-1], last_mm, sync=True,
                           reason="observe PE on SP")
            nc.sync.drain()
            add_dep_helper(nc.cur_bb.bb.instructions[-1], last_act, sync=True,
                           reason="observe ACT on SP")
            nc.sync.drain()
            add_dep_helper(nc.cur_bb.bb.instructions[-1], agin_dma, sync=True,
                           reason="observe gpsimd DMA queue on SP")
            nc.sync.drain()
            add_dep_helper(nc.cur_bb.bb.instructions[-1], cc_inst, sync=True,
                           reason="observe collective on SP")
            nc.sync.drain()
            add_dep_helper(nc.cur_bb.bb.instructions[-1], asel_inst, sync=True,
                           reason="observe gpsimd engine on SP")
            for g in gather_dmas:
                nc.sync.drain()
                add_dep_helper(nc.cur_bb.bb.instructions[-1], g, sync=True,
                               reason="observe gather DMA queue on SP")
            nc.sync.dma_start(out=esums.ap()[:, :], in_=esums_s[:, :])
            out_dma = nc.cur_bb.bb.instructions[-1]
            nc.sync.drain()
            add_dep_helper(nc.cur_bb.bb.instructions[-1], out_dma, sync=True,
                           reason="observe out DMA queue on SP")
    return nc


def _get_nc():
    if "nc" not in _CACHE:
        _CACHE["nc"] = _build_bass()
    return _CACHE["nc"]


def _host_inputs(emb):
    """Per-core in_maps: just the core's own [128, 1024] embT shard (fp8)."""
    import ml_dtypes
    return [{"eshard": emb[RPC * c:RPC * (c + 1)].T.astype(ml_dtypes.float8_e4m3)}
            for c in range(NCORES)]


def _get_exec():
    """Build ONCE a jitted shard_map executor for the bass program.

    run_bass_kernel_spmd re-creates and re-jits its closure on every call
    (fresh trace + lower each time); since the wall-clock metric is
    dominated by per-call dispatch, cache the jitted callable and reuse it.
    Mirrors bass2jax.run_bass_via_pjrt's multi-core path exactly.
    """
    if "exec" in _CACHE:
        return _CACHE["exec"]
    import jax
    from jax.sharding import Mesh, PartitionSpec
    import warnings
    with warnings.catch_warnings():
        warnings.simplefilter("ignore")
        from jax.experimental.shard_map import shard_map
    from concourse.bass2jax import (
        install_neuronx_cc_hook, _bass_exec_p, partition_id_tensor,
    )

    nc = _get_nc()
    install_neuronx_cc_hook()
    assert nc.dbg_addr is None
    partition_name = (nc.partition_id_tensor.name
                      if nc.partition_id_tensor else None)
    in_names, out_names, out_avals = [], [], []
    for alloc in nc.m.functions[0].allocations:
        if not isinstance(alloc, mybir.MemoryLocationSet):
            continue
        name = alloc.memorylocations[0].name
        if alloc.kind == "ExternalInput":
            if name != partition_name:
                in_names.append(name)
        elif alloc.kind == "ExternalOutput":
            out_names.append(name)
            out_avals.append(jax.core.ShapedArray(
                tuple(alloc.tensor_shape), mybir.dt.np(alloc.dtype)))
    assert in_names == ["eshard"] and out_names == ["esums"]
    n_params = len(in_names)
    in_names_full = in_names + out_names
    if partition_name is not None:
        in_names_full.append(partition_name)
    donate = tuple(range(n_params, n_params + len(out_names)))

    def _body(*args):
        operands = list(args)
        if partition_name is not None:
            operands.append(partition_id_tensor())
        return tuple(_bass_exec_p.bind(
            *operands,
            out_avals=tuple(out_avals),
            in_names=tuple(in_names_full),
            out_names=tuple(out_names),
            lowering_input_output_aliases=(),
            sim_require_finite=True,
            sim_require_nnan=True,
            nc=nc,
        ))

    devices = jax.devices()[:NCORES]
    assert len(devices) == NCORES
    mesh = Mesh(np.asarray(devices), ("core",))
    nio = n_params + len(out_names)
    sharded = jax.jit(
        shard_map(_body, mesh=mesh,
                  in_specs=(PartitionSpec("core"),) * nio,
                  out_specs=(PartitionSpec("core"),) * len(out_names),
                  check_rep=False),
        donate_argnums=donate, keep_unused=True,
    )
    try:
        import ml_dtypes
        sharded = sharded.lower(
            jax.ShapeDtypeStruct((NCORES * 128, RPC), ml_dtypes.float8_e4m3),
            jax.ShapeDtypeStruct((NCORES * 128, 2 * NT), np.float32),
        ).compile()
    except Exception:
        pass  # plain jit callable works too, just ~1 ms more dispatch
    _CACHE["exec"] = sharded
    return sharded


def _get_conv():
    """XLA-CPU jitted f32 -> fp8-e4m3 shard converter.

    ~2 ms vs ~5-6 ms for the numpy f16+LUT path, and bit-exact with
    ml_dtypes' round-to-nearest f32->e4m3 cast. TRN2 doesn't accept the
    e4m3fn HLO dtype, but the CPU backend does, and e4m3fn bytes equal
    e4m3 bytes for all finite values in our range (they differ only in
    the top binade / specials), so the result is viewed as e4m3.
    """
    if "conv" in _CACHE:
        return _CACHE["conv"]
    import jax
    import jax.numpy as jnp
    cpu = jax.devices("cpu")[0]
    with jax.default_device(cpu):
        @jax.jit
        def _c(e):
            s = e.reshape(NCORES, RPC, D).transpose(0, 2, 1)
            return s.astype(jnp.float8_e4m3fn).reshape(NCORES * 128, RPC)
        np.asarray(_c(np.zeros((B, D), np.float32)))  # warm the trace
    _CACHE["conv"] = (jax, cpu, _c)
    return _CACHE["conv"]


def _run_spmd_fallback(emb):
    """Reference path through run_bass_kernel_spmd (per-call re-jit)."""
    global last_results
    res = run_bass_kernel_spmd(_get_nc(), _host_inputs(emb),
                               core_ids=list(range(NCORES)))
    last_results = res
    return np.stack([np.asarray(res.results[c]["esums"])
                     for c in range(NCORES)])


def kernel(embeddings, labels):
    emb = np.ascontiguousarray(np.asarray(embeddings, dtype=np.float32))
    labels = np.asarray(labels).astype(np.int64)
    assert emb.shape == (B, D) and labels.shape == (B,)

    # stacked shards [8*128, 1024] fp8: rows [128c:128(c+1)] = shard c, T'd
    import ml_dtypes
    try:
        jx, cpu, cfun = _get_conv()
        with jx.default_device(cpu):
            stacked = np.asarray(cfun(emb)).view(ml_dtypes.float8_e4m3)
    except Exception:
        # numpy fallback: f32 -> f16 (round-to-nearest) -> e4m3 via a 64K
        # LUT on the f16 bits (~6 ms; only 1-ulp tie deviations)
        if "f8lut" not in _CACHE:
            with np.errstate(invalid="ignore", over="ignore"):
                _CACHE["f8lut"] = (np.arange(65536, dtype=np.uint16)
                                   .view(np.float16)
                                   .astype(ml_dtypes.float8_e4m3)
                                   .view(np.uint8))
        s16 = emb.reshape(NCORES, RPC, D).transpose(0, 2, 1).astype(np.float16)
        stacked = (_CACHE["f8lut"][s16.view(np.uint16)]
                   .view(ml_dtypes.float8_e4m3).reshape(NCORES * 128, RPC))
    out_fut = None
    try:
        sharded = _get_exec()
        out_fut = sharded(stacked, np.zeros((NCORES * 128, 2 * NT), np.float32))
    except Exception:
        outs3 = _run_spmd_fallback(emb)

    # overlap the label/einsum host math with the device round trip
    emb64 = emb.astype(np.float64)
    nclass = int(labels.max()) + 1
    cnt = np.bincount(labels, minlength=nclass)
    pc = cnt[labels] - 1                      # positives per row (excl. self)
    G = np.zeros((nclass, D), dtype=np.float64)
    np.add.at(G, labels, emb64)
    # q_i = sum over positives j (same label, j != i) of sim[i, j]
    q = (np.einsum("ij,ij->i", emb64, G[labels])
         - np.einsum("ij,ij->i", emb64, emb64)) / TEMP

    if out_fut is not None:
        outs3 = np.asarray(out_fut[0]).reshape(NCORES, 128, 2 * NT)

    # esums[p, 0:8] full exp-sum, [p, 8:16] exp(diag); local row = 128*t + p
    tot = np.ascontiguousarray(
        outs3[:, :, :NT].transpose(0, 2, 1)).reshape(-1).astype(np.float64)
    expd = np.ascontiguousarray(
        outs3[:, :, NT:].transpose(0, 2, 1)).reshape(-1).astype(np.float64)
    esum = tot - expd

    lse = np.log(esum)
    has = pc > 0
    row_mean = np.where(has, q / np.maximum(pc, 1) - lse, 0.0)
    loss = -row_mean.sum() / max(int(has.sum()), 1)
    return np.float32(loss)


# revision 38
# speedup vs baseline: 1.1872x; 1.1107x over previous
"""Contrastive loss (InfoNCE-style) on 8 Trainium2 NeuronCores.

Reference math (B=8192, D=128, temp=0.07):
    sim = (emb @ emb.T) / temp, diag masked to -1e9
    log_probs = log_softmax(sim, axis=1)
    row_mean_i = mean over positives (same label, j != i) of log_probs[i, :]
    loss = -sum(row_mean_i) / count(rows with >=1 positive)

Decomposition used here:
    log_probs[i, j] = sim[i, j] - lse_i,   lse_i = log(sum_{j!=i} exp(sim[i, j]))
    pos_sum_i  = q_i - pc_i * lse_i, where q_i = sum_{j pos} sim[i, j] (exact,
                 computed on host in f64 via class-summed embeddings) and
                 pc_i = (# rows with same label) - 1 (host, exact integer math)
    => the ONLY O(B^2) quantity is esum_i = sum_{j!=i} exp(sim[i, j]).

Perf model (vs the 1.0 s/call baseline): the wall-clock metric is dominated
by the axon tunnel, which has a ~70 ms fixed per-call round-trip cost plus
~16 ms/MB of input transfer; device compute is <1 ms. Three changes attack
exactly that:
  1. Each core receives ONLY its own [128, 1024] embT shard in fp8-e4m3
     (1 MB total, vs 38 MB of per-core rotated f32 copies); the full table
     is rebuilt ON DEVICE with a DRAM AllGather over NeuronLink.
  2. The jitted shard_map executor is built once and cached
     (run_bass_kernel_spmd re-traces + re-jits a fresh closure per call).
  3. The call is dispatched async and the label/einsum host math runs
     during the device round trip.
fp8 numerics: exp-arg jitter ~0.03 -> per-row lse error ~7e-4, final loss
rel err ~1e-4, vs the 2e-2 gate.

Device kernel (per core, SPMD-uniform, no rotation needed):
    - DMA own shard [128, 1024] -> SBUF (lhs source)
    - DRAM bounce + AllGather -> agout [8, 128, 1024]; one multi-dim DMA
      ([c,p,j] -> [p,c,j]) -> embT [128, 8192] in natural global order
    - self-blocks: per row-tile t, matmul lhs_t^T lhs_t -> diag holds raw
      s_ii; affine_select keeps the diagonal (fill -30000), ACT Exp accum
      -> expd[:, t] = exp(s_ii/temp), bit-identical to the diag term inside
      the main sum (same PE/ACT datapath on same operand bits)
    - main: per tile t, 4 quarters x 4 matmuls [128,512] (fp8 -> f32 PSUM),
      ACT Exp(in/temp) with accum_out -> per-quarter row sums
    - output esums [128, 16]: cols 0:8 total exp-sums (incl. self term),
      cols 8:16 exp(diag). Host: esum_excl = total - expd in f64 (exact).

Host: lse = log(esum_excl); row_mean = q/pc - lse (where pc>0); reduce.
"""

import numpy as np

import concourse.bass as bass
import concourse.mybir as mybir
import concourse.tile as tile
from concourse.tile import add_dep_helper
from concourse.bass_utils import run_bass_kernel_spmd

TEMP = 0.07
B = 8192
D = 128
NCORES = 8
RPC = B // NCORES        # 1024 rows per core
NT = RPC // 128          # 8 row-tiles of 128 rows per core
MASK_RAW = -30000.0      # raw-dot space; exp(MASK/temp) == 0.0 in f32

_CACHE = {}

# test.py introspection: last BassKernelResults from run_bass_kernel_spmd.
last_results = None


def _build_bass():
    f32 = mybir.dt.float32
    f8 = mybir.dt.float8e4
    bf16 = mybir.dt.bfloat16
    nc = bass.Bass("TRN2", target_bir_lowering=False, debug=False,
                   num_devices=NCORES)
    eshard = nc.dram_tensor("eshard", [128, RPC], f8, kind="ExternalInput")
    esums = nc.dram_tensor("esums", [128, 2 * NT], f32, kind="ExternalOutput")

    with tile.TileContext(nc) as tc:
        with (
            tc.tile_pool(name="big", bufs=1) as big,
            tc.tile_pool(name="psum", bufs=2, space="PSUM") as psum,
            tc.tile_pool(name="scratch", bufs=32) as scratch,
            tc.tile_pool(name="small", bufs=1) as small,
            tc.tile_pool(name="dram", bufs=1, space="DRAM") as dram,
        ):
            shard_s = big.tile([128, RPC], f8)
            nc.sync.dma_start(out=shard_s[:, :], in_=eshard.ap()[:, :])
            in_dma0 = nc.cur_bb.bb.instructions[-1]
            nc.sync.drain()
            add_dep_helper(nc.cur_bb.bb.instructions[-1], in_dma0, sync=True,
                           reason="observe input DMA queue on SP")

            # AllGather: input bounce (collectives can't touch I/O tensors),
            # gather to a Shared DRAM scratch, then one DMA rebuilds the
            # full [128, 8192] column table in SBUF in natural global order.
            agin = dram.tile([128, RPC], f8)
            agout = dram.tile([NCORES, 128, RPC], f8, addr_space="Shared")
            nc.gpsimd.dma_start(out=agin[:, :], in_=eshard.ap()[:, :])
            agin_dma = nc.cur_bb.bb.instructions[-1]
            nc.gpsimd.collective_compute(
                "AllGather", mybir.AluOpType.bypass,
                replica_groups=[list(range(NCORES))],
                ins=[agin.opt()], outs=[agout.opt()],
            )
            cc_inst = nc.cur_bb.bb.instructions[-1]
            embT = big.tile([128, B], f8)
            # ONE multi-dim DMA for all 8 gathered pieces: walking the DRAM
            # side [c, p, j] -> [p, c, j] lands piece c at SBUF columns
            # [1024c, 1024(c+1)). A single DMA keeps every queue at one
            # entry (walrus allows only one sync wait per DMA entry) and
            # carries the collective wait for the whole gather.
            nc.sync.dma_start(
                out=embT[:, :].rearrange("p (c j) -> p c j", c=NCORES),
                in_=agout[:, :, :].transpose([1, 0, 2]),
            )
            gather_dmas = [nc.cur_bb.bb.instructions[-1]]

            esum_all = small.tile([128, NT * 4], f32)
            esums_s = small.tile([128, 2 * NT], f32)

            # prefetch dummy: a discarded LDWEIGHTS observing the shard DMA,
            # so real matmuls don't carry that queue wait (walrus limit)
            nc.tensor.ldweights(shard_s[:, 0:2].bitcast(bf16))

            # --- self blocks (only need the own shard; overlaps the gather)
            ps_self = psum.tile([128, 2048], f32, tag="ps")
            for t in range(NT):
                lhs = shard_s[:, t * 128:(t + 1) * 128]
                nc.tensor.matmul(ps_self[:, t * 128:(t + 1) * 128], lhs, lhs,
                                 start=True, stop=True)
            # prefetch dummy: a discarded LDWEIGHTS observing the gather DMA
            # on PE, placed after the self matmuls so those still overlap
            # the collective; main matmuls then never carry the gather-queue
            # wait and stay within walrus's one-sync-wait limit
            nc.tensor.ldweights(embT[:, B - 2:B].bitcast(bf16))
            sb_all = small.tile([128, NT * 128], f32)
            nc.scalar.activation(sb_all[:, :], ps_self[:, 0:NT * 128],
                                 mybir.ActivationFunctionType.Copy)
            sbm = small.tile([128, NT * 128], f32)
            nc.gpsimd.affine_select(
                sbm[:, :], sb_all[:, :], pattern=[[0, NT], [-1, 128]],
                compare_op=mybir.AluOpType.is_equal, fill=MASK_RAW,
                base=0, channel_multiplier=1,
            )
            asel_inst = nc.cur_bb.bb.instructions[-1]
            # bf16 like the main-path scratch: the diag's output rounding then
            # matches the main sum's diag term bit-for-bit and cancels exactly
            junkd = small.tile([128, NT * 128], mybir.dt.bfloat16)
            for t in range(NT):
                nc.scalar.activation(
                    junkd[:, t * 128:(t + 1) * 128],
                    sbm[:, t * 128:(t + 1) * 128],
                    mybir.ActivationFunctionType.Exp, scale=1.0 / TEMP,
                    accum_out=esums_s[:, NT + t:NT + t + 1],
                )

            # --- main loop: 8 row-tiles x 4 quarters x 4 matmuls of [128,512]
            for t in range(NT):
                lhs = shard_s[:, t * 128:(t + 1) * 128]
                for q in range(4):
                    qi = t * 4 + q
                    a = qi + 1            # psum alloc index (ps_self was 0)
                    ps = psum.tile([128, 2048], f32, tag="ps")
                    carrier = None
                    if a >= 2:
                        # discarded LDWEIGHTS reading the 2-allocations-ago
                        # ACT result: carries the psum-WAR ACT wait so the
                        # slot-reuse matmul below carries only its own wait
                        obs = (sb_all[:, 0:1] if a == 2
                               else esum_all[:, a - 3:a - 2])
                        nc.tensor.ldweights(obs.bitcast(bf16))
                        carrier = nc.cur_bb.bb.instructions[-1]
                    for k in range(4):
                        n = 4 * q + k
                        nc.tensor.matmul(
                            ps[:, k * 512:(k + 1) * 512],
                            lhs,
                            embT[:, n * 512:(n + 1) * 512],
                            start=True, stop=True,
                        )
                        if carrier is not None:
                            add_dep_helper(nc.cur_bb.bb.instructions[-1],
                                           carrier, sync=False,
                                           reason="wait-carrier order")
                            carrier = None
                        last_mm = nc.cur_bb.bb.instructions[-1]
                    scr = scratch.tile([128, 2048], mybir.dt.bfloat16)
                    nc.scalar.activation(
                        scr[:, :], ps[:, :],
                        mybir.ActivationFunctionType.Exp,
                        scale=1.0 / TEMP,
                        accum_out=esum_all[:, qi:qi + 1],
                    )

            # final [128, 4] -> [128, 1] sums per row-tile on the scalar
            # engine (keeps the vector engine out of the program)
            junk = small.tile([128, 4 * NT], f32)
            for t in range(NT):
                nc.scalar.activation(
                    junk[:, t * 4:(t + 1) * 4],
                    esum_all[:, t * 4:(t + 1) * 4],
                    mybir.ActivationFunctionType.Copy,
                    accum_out=esums_s[:, t:t + 1],
                )
            last_act = nc.cur_bb.bb.instructions[-1]
            # one manual drain per outstanding proc, each carrying a single
            # wait, so the auto-generated kernel-tail drain (which tolerates
            # almost no sync waits) has nothing left to wait for
            nc.sync.drain()
            add_dep_helper(nc.cur_bb.bb.instructions[-1], last_mm, sync=True,
                           reason="observe PE on SP")
            nc.sync.drain()
            add_dep_helper(nc.cur_bb.bb.instructions[-1], last_act, sync=True,
                           reason="observe ACT on SP")
            nc.sync.drain()
            add_dep_helper(nc.cur_bb.bb.instructions[-1], agin_dma, sync=True,
                           reason="observe gpsimd DMA queue on SP")
            nc.sync.drain()
            add_dep_helper(nc.cur_bb.bb.instructions[-1], cc_inst, sync=True,
                           reason="observe collective on SP")
            nc.sync.drain()
            add_dep_helper(nc.cur_bb.bb.instructions[-1], asel_inst, sync=True,
                           reason="observe gpsimd engine on SP")
            for g in gather_dmas:
                nc.sync.drain()
                add_dep_helper(nc.cur_bb.bb.instructions[-1], g, sync=True,
                               reason="observe gather DMA queue on SP")
            nc.sync.dma_start(out=esums.ap()[:, :], in_=esums_s[:, :])
            out_dma = nc.cur_bb.bb.instructions[-1]
            nc.sync.drain()
            add_dep_helper(nc.cur_bb.bb.instructions[-1], out_dma, sync=True,
                           reason="observe out DMA queue on SP")
    return nc


def _get_nc():
    if "nc" not in _CACHE:
        _CACHE["nc"] = _build_bass()
    return _CACHE["nc"]


def _host_inputs(emb):
    """Per-core in_maps: just the core's own [128, 1024] embT shard (fp8)."""
    import ml_dtypes
    return [{"eshard": emb[RPC * c:RPC * (c + 1)].T.astype(ml_dtypes.float8_e4m3)}
            for c in range(NCORES)]


def _get_exec():
    """Build ONCE a jitted shard_map executor for the bass program.

    run_bass_kernel_spmd re-creates and re-jits its closure on every call
    (fresh trace + lower each time); since the wall-clock metric is
    dominated by per-call dispatch, cache the jitted callable and reuse it.
    Mirrors bass2jax.run_bass_via_pjrt's multi-core path exactly.
    """
    if "exec" in _CACHE:
        return _CACHE["exec"]
    import jax
    from jax.sharding import Mesh, PartitionSpec
    import warnings
    with warnings.catch_warnings():
        warnings.simplefilter("ignore")
        from jax.experimental.shard_map import shard_map
    from concourse.bass2jax import (
        install_neuronx_cc_hook, _bass_exec_p, partition_id_tensor,
    )

    nc = _get_nc()
    install_neuronx_cc_hook()
    assert nc.dbg_addr is None
    partition_name = (nc.partition_id_tensor.name
                      if nc.partition_id_tensor else None)
    in_names, out_names, out_avals = [], [], []
    for alloc in nc.m.functions[0].allocations:
        if not isinstance(alloc, mybir.MemoryLocationSet):
            continue
        name = alloc.memorylocations[0].name
        if alloc.kind == "ExternalInput":
            if name != partition_name:
                in_names.append(name)
        elif alloc.kind == "ExternalOutput":
            out_names.append(name)
            out_avals.append(jax.core.ShapedArray(
                tuple(alloc.tensor_shape), mybir.dt.np(alloc.dtype)))
    assert in_names == ["eshard"] and out_names == ["esums"]
    n_params = len(in_names)
    in_names_full = in_names + out_names
    if partition_name is not None:
        in_names_full.append(partition_name)
    donate = tuple(range(n_params, n_params + len(out_names)))

    def _body(*args):
        operands = list(args)
        if partition_name is not None:
            operands.append(partition_id_tensor())
        return tuple(_bass_exec_p.bind(
            *operands,
            out_avals=tuple(out_avals),
            in_names=tuple(in_names_full),
            out_names=tuple(out_names),
            lowering_input_output_aliases=(),
            sim_require_finite=True,
            sim_require_nnan=True,
            nc=nc,
        ))

    devices = jax.devices()[:NCORES]
    assert len(devices) == NCORES
    mesh = Mesh(np.asarray(devices), ("core",))
    nio = n_params + len(out_names)
    sharded = jax.jit(
        shard_map(_body, mesh=mesh,
                  in_specs=(PartitionSpec("core"),) * nio,
                  out_specs=(PartitionSpec("core"),) * len(out_names),
                  check_rep=False),
        donate_argnums=donate, keep_unused=True,
    )
    try:
        import ml_dtypes
        sharded = sharded.lower(
            jax.ShapeDtypeStruct((NCORES * 128, RPC), ml_dtypes.float8_e4m3),
            jax.ShapeDtypeStruct((NCORES * 128, 2 * NT), np.float32),
        ).compile()
    except Exception:
        pass  # plain jit callable works too, just ~1 ms more dispatch
    _CACHE["exec"] = sharded
    return sharded


def _get_post():
    """XLA-CPU jitted label math: (emb, labels) -> (q, pc).

    The numpy version (np.add.at + f64 einsums, ~30 ms of GIL-holding
    single-thread work) runs "overlapped" with the device round trip but
    competes with the axon client's CPU threads and stretches the round
    by ~14 ms. The XLA-CPU version is ~2-4 ms, multithreaded, and
    releases the GIL while executing.
    """
    if "post" in _CACHE:
        return _CACHE["post"]
    import jax
    import jax.numpy as jnp
    cpu = jax.devices("cpu")[0]
    NCLASS = 2048          # labels are randint(0, 2048) per the spec
    with jax.default_device(cpu):
        @jax.jit
        def _p(e, lab):
            cnt = jnp.zeros((NCLASS,), jnp.int32).at[lab].add(1)
            pc = cnt[lab] - 1
            G = jnp.zeros((NCLASS, D), jnp.float32).at[lab].add(e)
            q = (jnp.einsum("ij,ij->i", e, G[lab])
                 - jnp.einsum("ij,ij->i", e, e)) / TEMP
            return q, pc
        r = _p(np.zeros((B, D), np.float32), np.zeros((B,), np.int32))
        [np.asarray(v) for v in r]  # warm the trace
    _CACHE["post"] = (jax, cpu, _p)
    return _CACHE["post"]


def _get_conv():
    """XLA-CPU jitted f32 -> fp8-e4m3 shard converter.

    ~2 ms vs ~5-6 ms for the numpy f16+LUT path, and bit-exact with
    ml_dtypes' round-to-nearest f32->e4m3 cast. TRN2 doesn't accept the
    e4m3fn HLO dtype, but the CPU backend does, and e4m3fn bytes equal
    e4m3 bytes for all finite values in our range (they differ only in
    the top binade / specials), so the result is viewed as e4m3.
    """
    if "conv" in _CACHE:
        return _CACHE["conv"]
    import jax
    import jax.numpy as jnp
    cpu = jax.devices("cpu")[0]
    with jax.default_device(cpu):
        @jax.jit
        def _c(e):
            s = e.reshape(NCORES, RPC, D).transpose(0, 2, 1)
            return s.astype(jnp.float8_e4m3fn).reshape(NCORES * 128, RPC)
        np.asarray(_c(np.zeros((B, D), np.float32)))  # warm the trace
    _CACHE["conv"] = (jax, cpu, _c)
    return _CACHE["conv"]


def _run_spmd_fallback(emb):
    """Reference path through run_bass_kernel_spmd (per-call re-jit)."""
    global last_results
    res = run_bass_kernel_spmd(_get_nc(), _host_inputs(emb),
                               core_ids=list(range(NCORES)))
    last_results = res
    return np.stack([np.asarray(res.results[c]["esums"])
                     for c in range(NCORES)])


def kernel(embeddings, labels):
    emb = np.ascontiguousarray(np.asarray(embeddings, dtype=np.float32))
    labels = np.asarray(labels).astype(np.int64)
    assert emb.shape == (B, D) and labels.shape == (B,)

    # stacked shards [8*128, 1024] fp8: rows [128c:128(c+1)] = shard c, T'd
    import ml_dtypes
    try:
        jx, cpu, cfun = _get_conv()
        with jx.default_device(cpu):
            stacked = np.asarray(cfun(emb)).view(ml_dtypes.float8_e4m3)
    except Exception:
        # numpy fallback: f32 -> f16 (round-to-nearest) -> e4m3 via a 64K
        # LUT on the f16 bits (~6 ms; only 1-ulp tie deviations)
        if "f8lut" not in _CACHE:
            with np.errstate(invalid="ignore", over="ignore"):
                _CACHE["f8lut"] = (np.arange(65536, dtype=np.uint16)
                                   .view(np.float16)
                                   .astype(ml_dtypes.float8_e4m3)
                                   .view(np.uint8))
        s16 = emb.reshape(NCORES, RPC, D).transpose(0, 2, 1).astype(np.float16)
        stacked = (_CACHE["f8lut"][s16.view(np.uint16)]
                   .view(ml_dtypes.float8_e4m3).reshape(NCORES * 128, RPC))
    out_fut = None
    try:
        sharded = _get_exec()
        out_fut = sharded(stacked, np.zeros((NCORES * 128, 2 * NT), np.float32))
    except Exception:
        outs3 = _run_spmd_fallback(emb)

    # overlap the label/einsum host math with the device round trip;
    # q_i = sum over positives j (same label, j != i) of sim[i, j]
    try:
        jx2, cpu2, pfun = _get_post()
        assert int(labels.max()) < 2048 and int(labels.min()) >= 0
        with jx2.default_device(cpu2):
            qj, pcj = pfun(emb, labels.astype(np.int32))
            q = np.asarray(qj).astype(np.float64)
            pc = np.asarray(pcj)
    except Exception:
        emb64 = emb.astype(np.float64)
        nclass = int(labels.max()) + 1
        cnt = np.bincount(labels, minlength=nclass)
        pc = cnt[labels] - 1                  # positives per row (excl. self)
        G = np.zeros((nclass, D), dtype=np.float64)
        np.add.at(G, labels, emb64)
        q = (np.einsum("ij,ij->i", emb64, G[labels])
             - np.einsum("ij,ij->i", emb64, emb64)) / TEMP

    if out_fut is not None:
        outs3 = np.asarray(out_fut[0]).reshape(NCORES, 128, 2 * NT)

    # esums[p, 0:8] full exp-sum, [p, 8:16] exp(diag); local row = 128*t + p
    tot = np.ascontiguousarray(
        outs3[:, :, :NT].transpose(0, 2, 1)).reshape(-1).astype(np.float64)
    expd = np.ascontiguousarray(
        outs3[:, :, NT:].transpose(0, 2, 1)).reshape(-1).astype(np.float64)
    esum = tot - expd

    lse = np.log(esum)
    has = pc > 0
    row_mean = np.where(has, q / np.maximum(pc, 1) - lse, 0.0)
    loss = -row_mean.sum() / max(int(has.sum()), 1)
    return np.float32(loss)
